# revision 2
# baseline (speedup 1.0000x reference)
"""Single-head causal attention (B=8, T=2048, D=1024, H=128) on 8 TRN2 NeuronCores.

Sharding: one batch element per core (data-parallel over B).

Per-core algorithm (all big matmuls in float32r: full PE speed, ~1.5e-4 rel err):
  - host supplies x^T [D, T] so the d-contraction has d on partitions
  - Q^T, K^T = W^T @ x^T   [H=128, T] via PE, d-tile-outer to overlap with DMA
  - V^T likewise, then PE-transposed to V [T, H] tiles (PV needs k on partitions)
  - per 512-wide q-chunk: S^T[k, q] = K^T_tile.T @ Q^T_chunk, exp via ACT
    (no max-subtraction: scores are O(20) for this distribution, exp is safe in fp32),
    causal mask on diagonal tiles via shifted upper-tri mask multiply,
    O^T[h, q] += V_tile.T @ P^T accumulated over k-tiles in PSUM,
    row-sums via DVE adds of P^T tiles + ones-matmul, 1/sums broadcast via
    rank-1 matmul, final O^T * (1/sums) on DVE, DMA out.
  - host transposes O^T -> [T, H] per batch.
"""
import numpy as np

B, T, D, H = 8, 2048, 1024, 128
ND = D // 128      # 8 d-tiles
NTK = T // 128     # 16 k-tiles
NCH = T // 512     # 4 q-chunks
SCALE = float(H) ** -0.5

_CACHE = {}


def _build():
    import concourse.bass as bass  # noqa: F401
    from concourse import bacc
    import concourse.mybir as mybir
    import concourse.tile as tile
    from concourse.masks import make_identity

    f32 = mybir.dt.float32
    f32r = mybir.dt.float32r

    nc = bacc.Bacc("TRN2", target_bir_lowering=False)
    xt_d = nc.dram_tensor("xt", (D, T), f32r, kind="ExternalInput")
    wq_d = nc.dram_tensor("wq", (128, ND, H), f32r, kind="ExternalInput")
    wk_d = nc.dram_tensor("wk", (128, ND, H), f32r, kind="ExternalInput")
    wv_d = nc.dram_tensor("wv", (128, ND, H), f32r, kind="ExternalInput")
    ot_d = nc.dram_tensor("ot", (H, T), f32, kind="ExternalOutput")

    with tile.TileContext(nc) as tc:
        with (
            tc.tile_pool(name="sb", bufs=1) as sb,
            tc.tile_pool(name="ps", bufs=1, space="PSUM") as ps,
        ):
            # ---- loads ----
            xt = sb.tile([128, ND, T], f32r, tag="xt")
            # chunk-major subtile DMAs so chunk ch's projections start after ~2MB
            for ch in range(NCH):
                for d in range(ND):
                    nc.sync.dma_start(xt[:, d, ch * 512:(ch + 1) * 512],
                                      xt_d[d * 128:(d + 1) * 128, ch * 512:(ch + 1) * 512])
            wq = sb.tile([128, ND, H], f32r, tag="wq")
            wk = sb.tile([128, ND, H], f32r, tag="wk")
            wv = sb.tile([128, ND, H], f32r, tag="wv")
            nc.sync.dma_start(wq[:], wq_d[:])
            nc.sync.dma_start(wk[:], wk_d[:])
            nc.sync.dma_start(wv[:], wv_d[:])

            # ---- constants ----
            ident = sb.tile([128, 128], f32, tag="ident")
            make_identity(nc, ident[:])
            # mask M[k, col] = 1 iff col - k >= 384; U_m = M[:, (3-m)*128 : +512]
            m32 = sb.tile([128, 896], f32, tag="m32")
            nc.gpsimd.memset(m32[:], 1.0)
            nc.gpsimd.affine_select(
                out=m32[:], in_=m32[:],
                compare_op=mybir.AluOpType.is_ge, fill=0.0,
                base=-384, pattern=[[1, 896]], channel_multiplier=-1,
            )
            maskM = sb.tile([128, 896], f32r, tag="maskM")
            nc.vector.tensor_copy(maskM[:], m32[:])
            ones_c32 = sb.tile([128, 1], f32, tag="ones_c32")
            nc.gpsimd.memset(ones_c32[:], 1.0)
            ones_col = sb.tile([128, 1], f32r, tag="ones_col")
            nc.vector.tensor_copy(ones_col[:], ones_c32[:])
            ones_r32 = sb.tile([1, 128], f32, tag="ones_r32")
            nc.gpsimd.memset(ones_r32[:], 1.0)
            ones_row = sb.tile([1, 128], f32r, tag="ones_row")
            nc.vector.tensor_copy(ones_row[:], ones_r32[:])

            # ---- projections (d-tile outer so PE consumes DMA'd tiles as they land) ----
            qt = sb.tile([128, T], f32r, tag="qt")   # Q^T [h, t]
            kt = sb.tile([128, T], f32r, tag="kt")   # K^T [h, t]
            vt = sb.tile([128, T], f32, tag="vt")    # V^T staging
            v = sb.tile([128, NTK, H], f32r, tag="v")  # V [k, h] tiles

            def attention_chunk(c):
                otp = ps.tile([128, 512], f32, tag="otacc", bufs=2)
                pacc = sb.tile([128, 512], f32r, tag="pacc", bufs=2)
                nk = 4 * c + 4
                for j in range(nk):
                    diag = j >= 4 * c
                    m = j - 4 * c if diag else 0
                    lo = 128 * m  # valid q_local range [lo, 512)
                    stp = ps.tile([128, 512], f32, tag="big", bufs=4)
                    nc.tensor.matmul(
                        stp[:, lo:512],
                        kt[:, j * 128:(j + 1) * 128],
                        qt[:, c * 512 + lo:(c + 1) * 512],
                        start=True, stop=True,
                    )
                    pt = sb.tile([128, 512], f32r, tag="pt", bufs=6)
                    if diag:
                        praw = sb.tile([128, 512], f32r, tag="praw", bufs=3)
                        nc.scalar.activation(
                            praw[:, lo:512], stp[:, lo:512],
                            mybir.ActivationFunctionType.Exp, scale=SCALE)
                        nc.vector.tensor_mul(
                            pt[:, lo:512], praw[:, lo:512], maskM[:, 384:896 - lo])
                    else:
                        nc.scalar.activation(
                            pt[:], stp[:],
                            mybir.ActivationFunctionType.Exp, scale=SCALE)
                    nc.tensor.matmul(
                        otp[:, lo:512], v[:, j, :], pt[:, lo:512],
                        start=(j == 0), stop=(j == nk - 1),
                    )
                    with nc.allow_low_precision(reason="f32r softmax denominator"):
                        if j == 0:
                            nc.vector.tensor_copy(pacc[:], pt[:])
                        else:
                            nc.vector.tensor_add(pacc[:, lo:512], pacc[:, lo:512],
                                                 pt[:, lo:512])
                sums = ps.tile([1, 512], f32, tag="sums")
                nc.tensor.matmul(sums[:], ones_col[:], pacc[:], start=True, stop=True)
                recip = sb.tile([1, 512], f32r, tag="recip", bufs=2)
                with nc.allow_low_precision(reason="f32r softmax denominator"):
                    nc.vector.reciprocal(recip[:], sums[:])
                bc = ps.tile([128, 512], f32, tag="bcast")
                nc.tensor.matmul(bc[:], ones_row[:], recip[:], start=True, stop=True)
                bc_sb = sb.tile([128, 512], f32, tag="bcsb", bufs=2)
                nc.vector.tensor_copy(bc_sb[:], bc[:])
                ot_sb = sb.tile([128, 512], f32, tag="otsb", bufs=2)
                nc.vector.tensor_mul(ot_sb[:], otp[:], bc_sb[:])
                nc.sync.dma_start(ot_d[:, c * 512:(c + 1) * 512], ot_sb[:])


            # ---- fused chunk-major pipeline: proj(ch) -> transposes -> attention(ch)
            for ch in range(NCH):
                for w_sb, dst, dst_r in ((wv, vt, False), (wk, kt, True), (wq, qt, True)):
                    acc = ps.tile([128, 512], f32, tag="big", bufs=4, name=f"acc_{ch}")
                    for d in range(ND):
                        nc.tensor.matmul(
                            acc[:], w_sb[:, d, :],
                            xt[:, d, ch * 512:(ch + 1) * 512],
                            start=(d == 0), stop=(d == ND - 1),
                        )
                    nc.vector.tensor_copy(dst[:, ch * 512:(ch + 1) * 512], acc[:])
                for j in range(4 * ch, 4 * ch + 4):
                    tp = ps.tile([128, 128], f32, tag="otacc", bufs=2)
                    nc.tensor.transpose(tp[:], vt[:, j * 128:(j + 1) * 128], ident[:])
                    nc.vector.tensor_copy(v[:, j, :], tp[:])
                attention_chunk(ch)

    nc.compile()
    return nc


def _in_maps(x, W_Q, W_K, W_V):
    def warr(W):
        return np.ascontiguousarray(
            np.asarray(W, np.float32).reshape(ND, 128, H).transpose(1, 0, 2))

    wqr, wkr, wvr = warr(W_Q), warr(W_K), warr(W_V)
    x = np.asarray(x, np.float32)
    return [
        {"xt": np.ascontiguousarray(x[b].T), "wq": wqr, "wk": wkr, "wv": wvr}
        for b in range(B)
    ]


def _run(inputs, **kw):
    from concourse import bass_utils

    if "nc" not in _CACHE:
        _CACHE["nc"] = _build()
    return bass_utils.run_bass_kernel_spmd(
        _CACHE["nc"], _in_maps(**inputs), core_ids=list(range(B)), **kw)


def kernel(x, W_Q, W_K, W_V):
    res = _run({"x": x, "W_Q": W_Q, "W_K": W_K, "W_V": W_V})
    return np.stack([res.results[b]["ot"].T for b in range(B)]).astype(np.float32)



# revision 4
# speedup vs baseline: 1.3459x; 1.3459x over previous
"""Single-head causal attention (B=8, T=2048, D=1024, H=128) on 8 TRN2 NeuronCores.

Sharding: one batch element per core (data-parallel over B).

Per-core algorithm (bf16 inputs, fp32 PSUM accumulation):
  - host supplies x^T [D, T] and weights in bf16
  - Q^T, K^T = W^T @ x^T   [H=128, T] via PE (f32 PSUM, evac to bf16 SBUF),
    d-tile outer so PE consumes DMA'd tiles as they land
  - V^T likewise, then PE-transposed to V [T, H] bf16 tiles (PV needs k on
    partitions)
  - per 512-wide q-chunk: S^T[k, q] = K^T_tile.T @ Q^T_chunk (bf16 matmul),
    exp via ACT straight to bf16 (no max-subtraction: scores are O(6) for this
    distribution, exp is safe), causal mask only on the 128x128 diagonal
    sub-block via one shared upper-tri mask multiply,
    O^T[h, q] += V_tile.T @ P^T accumulated over k-tiles in PSUM,
    row-sums via DVE bf16 adds of P^T tiles + ones-matmul, 1/sums broadcast
    via rank-1 matmul, final O^T * (1/sums) on DVE, DMA out fp32.
  - host transposes O^T -> [T, H] per batch.
"""
import numpy as np

B, T, D, H = 8, 2048, 1024, 128
ND = D // 128      # 8 d-tiles
NTK = T // 128     # 16 k-tiles
NCH = T // 512     # 4 q-chunks
SCALE = float(H) ** -0.5

_CACHE = {}


def _build():
    import concourse.bass as bass  # noqa: F401
    from concourse import bacc
    import concourse.mybir as mybir
    import concourse.tile as tile
    from concourse.masks import make_identity

    f32 = mybir.dt.float32
    f32r = mybir.dt.float32r
    bf16 = mybir.dt.bfloat16

    nc = bacc.Bacc("TRN2", target_bir_lowering=False)
    xt_d = nc.dram_tensor("xt", (D, T), bf16, kind="ExternalInput")
    wq_d = nc.dram_tensor("wq", (128, ND, H), bf16, kind="ExternalInput")
    wk_d = nc.dram_tensor("wk", (128, ND, H), bf16, kind="ExternalInput")
    wv_d = nc.dram_tensor("wv", (128, ND, H), bf16, kind="ExternalInput")
    ot_d = nc.dram_tensor("ot", (H, T), f32, kind="ExternalOutput")

    with tile.TileContext(nc) as tc:
        with (
            tc.tile_pool(name="sb", bufs=1) as sb,
            tc.tile_pool(name="ps", bufs=1, space="PSUM") as ps,
        ):
            # ---- loads (weights first: LDWEIGHTS needs them earliest) ----
            wq = sb.tile([128, ND, H], bf16, tag="wq")
            wk = sb.tile([128, ND, H], bf16, tag="wk")
            wv = sb.tile([128, ND, H], bf16, tag="wv")
            nc.sync.dma_start(wv[:], wv_d[:])
            nc.sync.dma_start(wk[:], wk_d[:])
            nc.sync.dma_start(wq[:], wq_d[:])
            xt = sb.tile([128, ND, T], bf16, tag="xt")
            # chunk-major subtile DMAs so chunk ch's projections start after ~1MB
            for ch in range(NCH):
                for d in range(ND):
                    nc.sync.dma_start(xt[:, d, ch * 512:(ch + 1) * 512],
                                      xt_d[d * 128:(d + 1) * 128, ch * 512:(ch + 1) * 512])

            # ---- constants ----
            ident = sb.tile([128, 128], bf16, tag="ident")
            make_identity(nc, ident[:])
            # tri32[k, q] = 1 iff q >= k (same mask for every diagonal block)
            tri32 = sb.tile([128, 128], f32, tag="tri32")
            nc.gpsimd.memset(tri32[:], 1.0)
            nc.gpsimd.affine_select(
                out=tri32[:], in_=tri32[:],
                compare_op=mybir.AluOpType.is_ge, fill=0.0,
                base=0, pattern=[[1, 128]], channel_multiplier=-1,
            )
            trimask = sb.tile([128, 128], bf16, tag="trimask")
            nc.vector.tensor_copy(trimask[:], tri32[:])
            ones_c32 = sb.tile([128, 1], f32, tag="ones_c32")
            nc.gpsimd.memset(ones_c32[:], 1.0)
            ones_col = sb.tile([128, 1], bf16, tag="ones_col")
            nc.vector.tensor_copy(ones_col[:], ones_c32[:])
            ones_r32 = sb.tile([1, 128], f32, tag="ones_r32")
            nc.gpsimd.memset(ones_r32[:], 1.0)
            ones_row = sb.tile([1, 128], f32r, tag="ones_row")
            nc.vector.tensor_copy(ones_row[:], ones_r32[:])

            qt = sb.tile([128, T], bf16, tag="qt")   # Q^T [h, t]
            kt = sb.tile([128, T], bf16, tag="kt")   # K^T [h, t]
            v = sb.tile([128, NTK, H], bf16, tag="v")  # V [k, h] tiles

            def attention_chunk(c):
                otp = ps.tile([128, 512], f32, tag="otacc", bufs=2)
                pacc = sb.tile([128, 512], bf16, tag="pacc", bufs=2)
                nk = 4 * c + 4
                for j in range(nk):
                    diag = j >= 4 * c
                    m = j - 4 * c if diag else 0
                    lo = 128 * m  # valid q_local range [lo, 512)
                    stp = ps.tile([128, 512], f32, tag="big", bufs=4)
                    nc.tensor.matmul(
                        stp[:, lo:512],
                        kt[:, j * 128:(j + 1) * 128],
                        qt[:, c * 512 + lo:(c + 1) * 512],
                        start=True, stop=True,
                    )
                    pt = sb.tile([128, 512], bf16, tag="pt", bufs=6)
                    nc.scalar.activation(
                        pt[:, lo:512], stp[:, lo:512],
                        mybir.ActivationFunctionType.Exp, scale=SCALE)
                    if diag:
                        # zero the upper-left triangle of the diag block
                        nc.vector.tensor_mul(
                            pt[:, lo:lo + 128], pt[:, lo:lo + 128], trimask[:])
                    nc.tensor.matmul(
                        otp[:, lo:512], v[:, j, :], pt[:, lo:512],
                        start=(j == 0), stop=(j == nk - 1),
                    )
                    with nc.allow_low_precision(reason="bf16 softmax denominator"):
                        if j == 0:
                            nc.vector.tensor_copy(pacc[:], pt[:])
                        else:
                            nc.vector.tensor_add(pacc[:, lo:512], pacc[:, lo:512],
                                                 pt[:, lo:512])
                sums = ps.tile([1, 512], f32, tag="red")
                nc.tensor.matmul(sums[:], ones_col[:], pacc[:], start=True, stop=True)
                recip = sb.tile([1, 512], f32r, tag="recip", bufs=2)
                with nc.allow_low_precision(reason="f32r softmax denominator"):
                    nc.vector.reciprocal(recip[:], sums[:])
                bc = ps.tile([128, 512], f32, tag="red")
                nc.tensor.matmul(bc[:], ones_row[:], recip[:], start=True, stop=True)
                bc_sb = sb.tile([128, 512], f32, tag="bcsb", bufs=2)
                nc.vector.tensor_copy(bc_sb[:], bc[:])
                ot_sb = sb.tile([128, 512], f32, tag="otsb", bufs=2)
                nc.vector.tensor_mul(ot_sb[:], otp[:], bc_sb[:])
                nc.sync.dma_start(ot_d[:, c * 512:(c + 1) * 512], ot_sb[:])

            # ---- fused chunk-major pipeline: proj(ch) -> transposes -> attention(ch)
            for ch in range(NCH):
                vt = sb.tile([128, 512], bf16, tag="vt", bufs=2)  # V^T staging
                for w_sb, dst in ((wv, vt), (wk, kt), (wq, qt)):
                    acc = ps.tile([128, 512], f32, tag="big", bufs=4, name=f"acc_{ch}")
                    for d in range(ND):
                        nc.tensor.matmul(
                            acc[:], w_sb[:, d, :],
                            xt[:, d, ch * 512:(ch + 1) * 512],
                            start=(d == 0), stop=(d == ND - 1),
                        )
                    with nc.allow_low_precision(reason="bf16 qkv"):
                        if dst is vt:
                            nc.vector.tensor_copy(dst[:], acc[:])
                        else:
                            nc.vector.tensor_copy(dst[:, ch * 512:(ch + 1) * 512], acc[:])
                for jj in range(4):
                    j = 4 * ch + jj
                    tp = ps.tile([128, 128], bf16, tag="tp")
                    nc.tensor.transpose(tp[:], vt[:, jj * 128:(jj + 1) * 128], ident[:])
                    with nc.allow_low_precision(reason="bf16 v"):
                        nc.vector.tensor_copy(v[:, j, :], tp[:])
                attention_chunk(ch)

    nc.compile()
    return nc


def _in_maps(x, W_Q, W_K, W_V):
    import ml_dtypes

    bf16 = ml_dtypes.bfloat16

    def warr(W):
        return np.ascontiguousarray(
            np.asarray(W, np.float32).reshape(ND, 128, H).transpose(1, 0, 2)
        ).astype(bf16)

    wqr, wkr, wvr = warr(W_Q), warr(W_K), warr(W_V)
    x = np.asarray(x, np.float32)
    return [
        {"xt": np.ascontiguousarray(x[b].T).astype(bf16),
         "wq": wqr, "wk": wkr, "wv": wvr}
        for b in range(B)
    ]


def _run(inputs, **kw):
    from concourse import bass_utils

    if "nc" not in _CACHE:
        _CACHE["nc"] = _build()
    return bass_utils.run_bass_kernel_spmd(
        _CACHE["nc"], _in_maps(**inputs), core_ids=list(range(B)), **kw)


def kernel(x, W_Q, W_K, W_V):
    res = _run({"x": x, "W_Q": W_Q, "W_K": W_K, "W_V": W_V})
    return np.stack([res.results[b]["ot"].T for b in range(B)]).astype(np.float32)


# revision 9
# speedup vs baseline: 1.6761x; 1.2453x over previous
"""Single-head causal attention (B=8, T=2048, D=1024, H=128) on 8 TRN2 NeuronCores.

Sharding: one batch element per core (data-parallel over B).

Per-core algorithm (bf16 inputs, fp32 PSUM accumulation):
  - host supplies x^T [D, T] and weights in bf16
  - Q^T, K^T = W^T @ x^T [H=128, T] via PE (f32 PSUM, evac to bf16 SBUF)
  - V^T likewise, PE-transposed to V [T, H] bf16 tiles (4 transposes ->
    one PSUM tile -> one evac)
  - per 512-wide q-chunk: S^T[k, q] = K^T_tile.T @ Q^T_chunk (bf16),
    exp via ACT straight to bf16 (scores are O(6), exp safe without
    max-subtraction), causal mask on the 128x128 diagonal block only,
    O^T[h, q] += V_tile.T @ P^T accumulated in PSUM,
    row-sums via DVE bf16 adds + ones-matmul.
  - chunk tails (recip/broadcast/normalize/DMA-out) are emitted AFTER the
    next chunk's projections so the slow ops never stall the in-order PE
    queue (software pipelining).
  - host transposes O^T -> [T, H] per batch.
"""
import numpy as np

B, T, D, H = 8, 2048, 1024, 128
ND = D // 128      # 8 d-tiles
NTK = T // 128     # 16 k-tiles
NCH = T // 512     # 4 q-chunks
SCALE = float(H) ** -0.5

_CACHE = {}


def _build():
    import concourse.bass as bass  # noqa: F401
    from concourse import bacc
    import concourse.mybir as mybir
    import concourse.tile as tile
    from concourse.masks import make_identity

    f32 = mybir.dt.float32
    bf16 = mybir.dt.bfloat16

    nc = bacc.Bacc("TRN2", target_bir_lowering=False)
    # xt[p, n, t] = x[b].T[n*128 + p, t] — partition-major so a whole chunk
    # can stream with one 3D DMA
    xt_d = nc.dram_tensor("xt", (128, ND, T), bf16, kind="ExternalInput")
    wq_d = nc.dram_tensor("wq", (128, ND, H), bf16, kind="ExternalInput")
    wk_d = nc.dram_tensor("wk", (128, ND, H), bf16, kind="ExternalInput")
    wv_d = nc.dram_tensor("wv", (128, ND, H), bf16, kind="ExternalInput")
    ot_d = nc.dram_tensor("ot", (H, T), f32, kind="ExternalOutput")

    with tile.TileContext(nc) as tc:
        with (
            tc.tile_pool(name="sb", bufs=1) as sb,
            tc.tile_pool(name="ps", bufs=1, space="PSUM") as ps,
        ):
            # ---- loads (weights first: LDWEIGHTS needs them earliest) ----
            wq = sb.tile([128, ND, H], bf16, tag="wq")
            wk = sb.tile([128, ND, H], bf16, tag="wk")
            wv = sb.tile([128, ND, H], bf16, tag="wv")
            nc.sync.dma_start(wv[:], wv_d[:])
            nc.sync.dma_start(wk[:], wk_d[:])
            nc.sync.dma_start(wq[:], wq_d[:])
            xt = sb.tile([128, ND, T], bf16, tag="xt")
            # chunk 0 at d-tile granularity (compute starts asap), rest coarse
            for d in range(ND):
                nc.sync.dma_start(xt[:, d, 0:512], xt_d[:, d, 0:512])
            for ch in range(1, NCH):
                nc.sync.dma_start(xt[:, :, ch * 512:(ch + 1) * 512],
                                  xt_d[:, :, ch * 512:(ch + 1) * 512])

            # ---- constants ----
            ident = sb.tile([128, 128], bf16, tag="ident")
            make_identity(nc, ident[:])
            # tri32[k, q] = 1 iff q >= k (same mask for every diagonal block)
            tri32 = sb.tile([128, 128], f32, tag="tri32")
            nc.gpsimd.memset(tri32[:], 1.0)
            nc.gpsimd.affine_select(
                out=tri32[:], in_=tri32[:],
                compare_op=mybir.AluOpType.is_ge, fill=0.0,
                base=0, pattern=[[1, 128]], channel_multiplier=-1,
            )
            trimask = sb.tile([128, 128], bf16, tag="trimask")
            nc.vector.tensor_copy(trimask[:], tri32[:])
            ones_c32 = sb.tile([128, 1], f32, tag="ones_c32")
            nc.gpsimd.memset(ones_c32[:], 1.0)
            ones_col = sb.tile([128, 1], bf16, tag="ones_col")
            nc.vector.tensor_copy(ones_col[:], ones_c32[:])
            # warm the ACT exp table while DMA streams in
            warm = sb.tile([128, 1], bf16, tag="warm")
            nc.scalar.activation(warm[:], ones_c32[:],
                                 mybir.ActivationFunctionType.Exp, scale=1.0)

            qt = sb.tile([128, T], bf16, tag="qt")   # Q^T [h, t]
            kt = sb.tile([128, T], bf16, tag="kt")   # K^T [h, t]
            v = sb.tile([128, NTK, H], bf16, tag="v")  # V [k, h] tiles

            def body(c):
                """S/exp/PV/pacc loop + row sums for chunk c."""
                otp = ps.tile([128, 512], f32, tag="otacc", bufs=2)
                pacc = sb.tile([128, 512], bf16, tag="pacc", bufs=2)
                nk = 4 * c + 4
                for j in range(nk):
                    diag = j >= 4 * c
                    m = j - 4 * c if diag else 0
                    lo = 128 * m  # valid q_local range [lo, 512)
                    stp = ps.tile([128, 512], f32, tag="big", bufs=4)
                    nc.tensor.matmul(
                        stp[:, lo:512],
                        kt[:, j * 128:(j + 1) * 128],
                        qt[:, c * 512 + lo:(c + 1) * 512],
                        start=True, stop=True,
                    )
                    pt = sb.tile([128, 512], bf16, tag="pt", bufs=6)
                    nc.scalar.activation(
                        pt[:, lo:512], stp[:, lo:512],
                        mybir.ActivationFunctionType.Exp, scale=SCALE)
                    if diag:
                        # zero the upper-left triangle of the diag block
                        nc.vector.tensor_mul(
                            pt[:, lo:lo + 128], pt[:, lo:lo + 128], trimask[:])
                    nc.tensor.matmul(
                        otp[:, lo:512], v[:, j, :], pt[:, lo:512],
                        start=(j == 0), stop=(j == nk - 1),
                    )
                    with nc.allow_low_precision(reason="bf16 softmax denominator"):
                        if j == 0:
                            nc.vector.tensor_copy(pacc[:], pt[:])
                        else:
                            nc.vector.tensor_add(pacc[:, lo:512], pacc[:, lo:512],
                                                 pt[:, lo:512])
                sums = ps.tile([1, 512], f32, tag="red")
                nc.tensor.matmul(sums[:], ones_col[:], pacc[:], start=True, stop=True)
                return otp, sums

            def tail(c, otp, sums):
                """normalize + DMA out for chunk c (emitted late: overlaps
                the next chunk's projections, so nothing here stalls PE)."""
                recip = sb.tile([1, 512], f32, tag="recip", bufs=2)
                nc.vector.reciprocal_approx_fast(out=recip[:], in_=sums[:])
                bc_sb = sb.tile([128, 512], f32, tag="bcsb", bufs=2)
                nc.gpsimd.partition_broadcast(bc_sb[:], recip[:])
                ot_sb = sb.tile([128, 512], f32, tag="otsb", bufs=2)
                nc.vector.tensor_mul(ot_sb[:], otp[:], bc_sb[:])
                nc.sync.dma_start(ot_d[:, c * 512:(c + 1) * 512], ot_sb[:])

            # ---- chunk-major pipeline, tails deferred one chunk ----
            pend = None
            for ch in range(NCH):
                vt = sb.tile([128, 512], bf16, tag="vt", bufs=2)  # V^T staging
                for w_sb, dst in ((wv, vt), (wk, kt), (wq, qt)):
                    acc = ps.tile([128, 512], f32, tag="big", bufs=4, name=f"acc_{ch}")
                    for d in range(ND):
                        nc.tensor.matmul(
                            acc[:], w_sb[:, d, :],
                            xt[:, d, ch * 512:(ch + 1) * 512],
                            start=(d == 0), stop=(d == ND - 1),
                        )
                    with nc.allow_low_precision(reason="bf16 qkv"):
                        if dst is vt:
                            nc.vector.tensor_copy(dst[:], acc[:])
                        else:
                            nc.vector.tensor_copy(dst[:, ch * 512:(ch + 1) * 512], acc[:])
                # 4 transposes -> one PSUM tile -> one evac
                tp = ps.tile([128, 512], bf16, tag="tp")
                for jj in range(4):
                    nc.tensor.transpose(tp[:, jj * 128:(jj + 1) * 128],
                                        vt[:, jj * 128:(jj + 1) * 128], ident[:])
                with nc.allow_low_precision(reason="bf16 v"):
                    nc.vector.tensor_copy(v[:, 4 * ch:4 * ch + 4, :], tp[:])
                if pend is not None:
                    tail(*pend)
                otp, sums = body(ch)
                pend = (ch, otp, sums)
            tail(*pend)

    nc.compile()
    return nc


def _in_maps(x, W_Q, W_K, W_V):
    import ml_dtypes

    bf16 = ml_dtypes.bfloat16

    def warr(W):
        return np.ascontiguousarray(
            np.asarray(W, np.float32).reshape(ND, 128, H).transpose(1, 0, 2)
        ).astype(bf16)

    wqr, wkr, wvr = warr(W_Q), warr(W_K), warr(W_V)
    x = np.asarray(x, np.float32)
    return [
        {"xt": np.ascontiguousarray(
            x[b].T.reshape(ND, 128, T).transpose(1, 0, 2)).astype(bf16),
         "wq": wqr, "wk": wkr, "wv": wvr}
        for b in range(B)
    ]


def _run(inputs, **kw):
    from concourse import bass_utils

    if "nc" not in _CACHE:
        _CACHE["nc"] = _build()
    return bass_utils.run_bass_kernel_spmd(
        _CACHE["nc"], _in_maps(**inputs), core_ids=list(range(B)), **kw)


def kernel(x, W_Q, W_K, W_V):
    res = _run({"x": x, "W_Q": W_Q, "W_K": W_K, "W_V": W_V})
    return np.stack([res.results[b]["ot"].T for b in range(B)]).astype(np.float32)
